# revision 19
# baseline (speedup 1.0000x reference)
"""Trainium2 Bass kernel: batched Piola-Kirchhoff stress P = dW/dF.

Per-sample closed-form gradient of
  W = 8*I1 + 10*J^2 - 56*log(J) + 0.2*(I4^2 + I5^2) - 44
with C = F^T (elementwise*) F, G = diag(4, .5, .5):
  P = s*cof(F) + I1/I4 diag terms + I5 terms,  s = 20*J - 56/J.

Data-parallel over 8 NeuronCores. Host passes each core a PLANE-MAJOR
shard [9, S] so every on-chip access is unit-stride (strided DVE reads
are ~16x slower). Plane order sigma = [a,b,e,f,i,g,c,h,d] lets shifted
tensor_tensor ops compute 16 of the 18 minor products in 5 instructions.
"""
import sys, types

sys.path.insert(0, "/opt/trn_rl_repo")

import numpy as np
import concourse.bass as bass
import concourse.mybir as mybir
from concourse import tile
from concourse.vector_clock import ScopedClock
from concourse.bass_utils import run_bass_kernel_spmd

AF = mybir.ActivationFunctionType
OP = mybir.AluOpType
FP32 = mybir.dt.float32

N = 4_000_000
N_CORES = 8
P = 128
L = 489          # samples per partition per tile
NT = 8           # tiles per core
R = L * NT       # 3912 samples per partition
S = P * R        # 500,736 samples per core (padded total 4,005,888)
LN56 = float(np.log(56.0))

# slot k of the on-chip X tile holds input plane SIG[k] (row-major a..i)
#        a  b  e  f  i  g  c  h  d
SIG = [0, 1, 4, 5, 8, 6, 2, 7, 3]

# ---------------------------------------------------------------- patches


def _patch_tile():
    if getattr(tile.TileContext, "_pk_patched", False):
        return

    def patched(self, tick_clock, wait_clock):
        drain_inst = self.nc.sync.drain()
        wait_clock.add_sem_waits(
            drain_inst.ins, ScopedClock({None: tick_clock.global_clock})
        )
        si = drain_inst.ins.sync_info
        if si is not None and len(si.on_wait) > 1:
            waits = list(si.on_wait)
            drain_inst.ins.sync_info = mybir.SyncInfo(
                on_wait=[waits[0]], on_update=list(si.on_update)
            )
            for w in waits[1:]:
                extra = self.nc.sync.drain()
                extra.ins.sync_info = mybir.SyncInfo(on_wait=[w], on_update=[])
        self.nc.all_engine_barrier()
        popped = self.nc._tile_sem_poison_stack.pop()
        assert popped is self._sem_poison
        self.nc.clear_and_free_semaphores(list(self.sems.allocated().values()))
        self.nc.all_engine_barrier()

    tile.TileContext._drain_and_barrier = patched

    # walrus accepts at most ONE sem wait per instruction: hoist extras onto
    # standalone EventSemaphore ops just before, on the same engine.
    orig_lower = tile.TileContext._lower_ordered_insts

    def lower_split(self, ordered):
        nc = self.nc
        for bb_name, insts in list(ordered.items()):
            out = []
            for inst in insts:
                si = inst.sync_info
                if si is not None and len(si.on_wait) > 1:
                    waits = list(si.on_wait)
                    for w in waits[:-1]:
                        ev = mybir.InstEventSemaphore(
                            name=nc.get_next_instruction_name(), ins=[], outs=[]
                        )
                        ev.engine = inst.engine
                        ev.sync_info = mybir.SyncInfo(on_wait=[w], on_update=[])
                        out.append(ev)
                    inst.sync_info = mybir.SyncInfo(
                        on_wait=[waits[-1]], on_update=list(si.on_update)
                    )
                out.append(inst)
            ordered[bb_name] = out
        return orig_lower(self, ordered)

    tile.TileContext._lower_ordered_insts = lower_split
    tile.TileContext._pk_patched = True


def _install_ntff_hook():
    try:
        import antenv
        if "antenv.axon_hooks" not in sys.modules:
            mod = types.ModuleType("antenv.axon_hooks")
            mod._hook = None
            mod.set_axon_ntff_profile_hook = lambda h: setattr(mod, "_hook", h)
            mod.get_axon_ntff_profile_hook = lambda: mod._hook
            sys.modules["antenv.axon_hooks"] = mod
            antenv.axon_hooks = mod
        from trn_agent_boot.trn_boot import _ntff_profile_via_ctypes
        sys.modules["antenv.axon_hooks"].set_axon_ntff_profile_hook(
            _ntff_profile_via_ctypes("/opt/axon/libaxon_pjrt.so")
        )
        import concourse.bass_utils as bu
        bu.upload_artifacts = lambda tmpdir: tmpdir
    except Exception as e:
        print(f"ntff hook install failed: {e}", file=sys.stderr)


# ---------------------------------------------------------------- program




def _b3(ap2d, nplanes):
    """[P, L] view of a [P,1,L] tile -> broadcast AP [P, nplanes, L]."""
    t = ap2d.tensor
    return t[:, :, :].to_broadcast((P, nplanes, L))

def _build_nc():
    _patch_tile()
    nc = bass.Bass()
    _c = nc.alloc_sbuf_tensor("const-ln56", [128, 1], FP32)
    nc.gpsimd.memset(_c.ap(), LN56)
    nc.const_aps.aps[(FP32, LN56)] = _c.ap()
    nc.all_engine_barrier()

    # pre-tiled host layout: [NT, 128, 9, L] (X-slot plane order baked on host)
    Fd = nc.dram_tensor("F", [NT, P, 9, L], FP32, kind="ExternalInput")
    Od = nc.dram_tensor("out", [NT, P, 9, L], FP32, kind="ExternalOutput")
    Fp, Op = Fd, Od

    with tile.TileContext(nc) as tc:
        with (
            tc.tile_pool(name="io", bufs=2) as iop,
            tc.tile_pool(name="mid", bufs=1) as midp,
            tc.tile_pool(name="mid2", bufs=1) as midp2,
            tc.tile_pool(name="sm", bufs=1) as smp,
        ):
            for t in range(NT):
                _emit_tile(nc, iop, midp, midp2, smp, Fp, Op, t)
    return nc


def _emit_tile(nc, iop, midp, midp2, smp, Fp, Op, t):
    mul, add, sub = OP.mult, OP.add, OP.subtract
    sl = slice(t * L, (t + 1) * L)

    X = iop.tile([P, 9, L], FP32, name="x", tag="x")
    nc.sync.dma_start(out=X[:, :, :], in_=Fp[t, :, :, :])

    def mtile(name, planes):
        return midp.tile([P, planes, L], FP32, name=name, tag=name)

    def stile(name):
        t = smp.tile([P, 1, L], FP32, name=name, tag=name)
        return t[:, 0, :]

    V = nc.vector

    # --- 18 minor products (shift-batched), PROD slots:
    # 0:cg 1:ch 2:dh | 3:ae 4:bf 5:ei 6:fg | 7:cd | 8:af 9:bi 10:eg |
    # 11:ai 12:bg 13:ce 14:fh 15:di | 16:ah 17:bd
    PR = midp2.tile([P, 18, L], FP32, name="prod", tag="prod")
    V.tensor_tensor(out=PR[:, 0:3, :], in0=X[:, 5:8, :], in1=X[:, 6:9, :], op=mul)
    V.tensor_tensor(out=PR[:, 3:7, :], in0=X[:, 0:4, :], in1=X[:, 2:6, :], op=mul)
    V.tensor_tensor(out=PR[:, 7, :], in0=X[:, 6, :], in1=X[:, 8, :], op=mul)
    V.tensor_tensor(out=PR[:, 8:11, :], in0=X[:, 0:3, :], in1=X[:, 3:6, :], op=mul)
    V.tensor_tensor(out=PR[:, 11:16, :], in0=X[:, 0:5, :], in1=X[:, 4:9, :], op=mul)
    V.tensor_tensor(out=PR[:, 16:18, :], in0=X[:, 0:2, :], in1=X[:, 7:9, :], op=mul)

    # early copies of late-needed products so PROD can retire after cofactors
    BDCF = mtile("bdcf", 3)          # (bd, cg, 8*fh)
    nc.scalar.activation(out=BDCF[:, 0, :], in_=PR[:, 17, :], func=AF.Copy)
    nc.scalar.activation(out=BDCF[:, 1, :], in_=PR[:, 0, :], func=AF.Copy)
    nc.scalar.activation(out=BDCF[:, 2, :], in_=PR[:, 14, :], func=AF.Copy, scale=8.0)

    # --- cofactors, row-major: 0:c00 1:c01 2:c02 3:c10 4:c11 5:c12 6:c20 7:c21 8:c22
    C = mtile("cof", 9)
    # C order: 0:c00 1:c01 2:c10 3:c02 4:c11 5:c12 6:c20 7:c21 8:c22
    V.tensor_tensor(out=C[:, 0:2, :], in0=PR[:, 5:7, :], in1=PR[:, 14:16, :], op=sub)
    V.tensor_tensor(out=C[:, 2:4, :], in0=PR[:, 1:3, :], in1=PR[:, 9:11, :], op=sub)
    V.tensor_tensor(out=C[:, 4, :], in0=PR[:, 11, :], in1=PR[:, 0, :], op=sub)
    V.tensor_tensor(out=C[:, 5, :], in0=PR[:, 12, :], in1=PR[:, 16, :], op=sub)
    V.tensor_tensor(out=C[:, 6, :], in0=PR[:, 4, :], in1=PR[:, 13, :], op=sub)
    V.tensor_tensor(out=C[:, 7, :], in0=PR[:, 7, :], in1=PR[:, 8, :], op=sub)
    V.tensor_tensor(out=C[:, 8, :], in0=PR[:, 3, :], in1=PR[:, 17, :], op=sub)

    # --- J = a*c00 + b*c01 + c*c02
    TAB = mtile("tab", 2)
    V.tensor_tensor(out=TAB[:, :, :], in0=X[:, 0:2, :], in1=C[:, 0:2, :], op=mul)
    tcp = stile("tcp")
    V.tensor_tensor(out=tcp, in0=X[:, 6, :], in1=C[:, 3, :], op=mul)
    j1 = stile("j1")
    V.tensor_tensor(out=j1, in0=TAB[:, 0, :], in1=TAB[:, 1, :], op=add)
    J = stile("J")
    V.tensor_tensor(out=J, in0=j1, in1=tcp, op=add)

    # --- s = 20*J - 56/J  (56/J = exp(-lnJ + ln56) on ACT)
    lnJ = stile("lnJ")
    nc.scalar.activation(out=lnJ, in_=J, func=AF.Ln, scale=1.0, bias=0.0)
    rec56 = stile("rec56")
    nc.scalar.activation(out=rec56, in_=lnJ, func=AF.Exp, scale=-1.0, bias=LN56)
    s = stile("s")
    V.scalar_tensor_tensor(out=s, in0=J, scalar=20.0, in1=rec56, op0=mul, op1=sub)

    # --- I5: q5x = 8*(ei^2-fh^2) + (ai^2-cg^2) + (ae^2-bd^2);  0.2*q5x = 0.4*I5
    SQ1 = mtile("sq1", 4)   # (ae2, bf2, ei2, fg2)
    nc.scalar.activation(out=SQ1[:, :, :], in_=PR[:, 3:7, :], func=AF.Square)
    SQ2 = mtile("sq2", 4)   # (ai2, bg2, ce2, fh2)
    nc.scalar.activation(out=SQ2[:, :, :], in_=PR[:, 11:15, :], func=AF.Square)
    SQBC = mtile("sqbc", 2)          # (sq_bd, sq_cg)
    nc.scalar.activation(out=SQBC[:, :, :], in_=BDCF[:, 0:2, :], func=AF.Square)
    sq_bd = SQBC[:, 0, :]
    sq_cg = SQBC[:, 1, :]
    t0 = stile("t0")
    V.tensor_tensor(out=t0, in0=SQ1[:, 2, :], in1=SQ2[:, 3, :], op=sub)
    t1 = stile("t1")
    V.tensor_tensor(out=t1, in0=SQ2[:, 0, :], in1=sq_cg, op=sub)
    t2 = stile("t2")
    V.tensor_tensor(out=t2, in0=SQ1[:, 0, :], in1=sq_bd, op=sub)
    u = stile("u")
    V.tensor_tensor(out=u, in0=t1, in1=t2, op=add)
    q5x = stile("q5x")
    V.scalar_tensor_tensor(out=q5x, in0=t0, scalar=8.0, in1=u, op0=mul, op1=add)

    # --- diag helpers: squares of a,e,i  (X slots 0,2,4)
    SQX = mtile("sqx", 5)   # (a2, b2, e2, f2, i2)
    nc.scalar.activation(out=SQX[:, :, :], in_=X[:, 0:5, :], func=AF.Square)
    YS = mtile("ys", 3)     # (y, y11, y22)
    V.tensor_tensor(out=YS[:, 0, :], in0=SQX[:, 2, :], in1=SQX[:, 4, :], op=add)
    V.scalar_tensor_tensor(out=YS[:, 1, :], in0=SQX[:, 4, :], scalar=8.0,
                           in1=SQX[:, 0, :], op0=mul, op1=add)
    V.scalar_tensor_tensor(out=YS[:, 2, :], in0=SQX[:, 2, :], scalar=8.0,
                           in1=SQX[:, 0, :], op0=mul, op1=add)
    I4x = stile("I4x")
    V.scalar_tensor_tensor(out=I4x, in0=SQX[:, 0, :], scalar=8.0,
                           in1=YS[:, 0, :], op0=mul, op1=add)

    # --- v = (0.2*ys)*q5x ; w = wI4*I4x + v ; za = (w+16)*(a,e,i)
    q5x3 = _b3(q5x, 3)
    V.scalar_tensor_tensor(out=YS[:, :, :], in0=YS[:, :, :], scalar=0.2,
                           in1=q5x3, op0=mul, op1=mul)
    V.scalar_tensor_tensor(out=YS[:, 0, :], in0=I4x, scalar=1.6,
                           in1=YS[:, 0, :], op0=mul, op1=add)
    I4x2 = _b3(I4x, 2)
    V.scalar_tensor_tensor(out=YS[:, 1:3, :], in0=I4x2, scalar=0.2,
                           in1=YS[:, 1:3, :], op0=mul, op1=add)
    ZA = YS
    V.scalar_tensor_tensor(out=ZA[:, :, :], in0=ZA[:, :, :], scalar=16.0,
                           in1=X[:, 0:5:2, :], op0=add, op1=mul)

    # --- OUT = s * cof (all 9), finals applied in place
    OUT = iop.tile([P, 9, L], FP32, name="outt", tag="outt")
    s9 = _b3(s, 9)
    V.tensor_tensor(out=OUT[:, :, :], in0=s9, in1=C[:, :, :], op=mul)
    # diag: OUT[0,4,8] += ZA
    V.tensor_tensor(out=OUT[:, 0:9:4, :], in0=OUT[:, 0:9:4, :], in1=ZA[:, :, :], op=add)

    # --- off-diagonals
    Q3 = mtile("q3", 3)              # (q5bd, q5cg, q5fh8) = 0.2*q5x*(bd, cg, 8fh)
    q5x3b = _b3(q5x, 3)
    V.scalar_tensor_tensor(out=Q3[:, :, :], in0=q5x3b, scalar=0.2,
                           in1=BDCF[:, :, :], op0=mul, op1=mul)
    q5bd = Q3[:, 0, :]
    q5cg = Q3[:, 1, :]
    q5fh8 = Q3[:, 2, :]

    MA = mtile("ma", 2)     # (m01, m10) = q5bd * (d, b)
    V.tensor_tensor(out=MA[:, 0, :], in0=q5bd, in1=X[:, 8, :], op=mul)   # *d
    V.tensor_tensor(out=MA[:, 1, :], in0=q5bd, in1=X[:, 1, :], op=mul)   # *b
    M2 = mtile("m2", 2)     # (m02, m20) = q5cg * (g, c) ; g=X5 c=X6
    q5cg2 = Q3[:, 1:2, :].to_broadcast((P, 2, L))
    V.tensor_tensor(out=M2[:, :, :], in0=q5cg2, in1=X[:, 5:7, :], op=mul)
    M3 = mtile("m3", 2)     # (m12, m21) = q5fh8 * (h, f)
    V.tensor_tensor(out=M3[:, 0, :], in0=q5fh8, in1=X[:, 7, :], op=mul)  # *h
    V.tensor_tensor(out=M3[:, 1, :], in0=q5fh8, in1=X[:, 3, :], op=mul)  # *f

    # P01,P10 -> OUT[1:3]; P02@3, P20@6 stride 3; P12@5, P21@7 stride 2
    V.tensor_tensor(out=OUT[:, 1:3, :], in0=OUT[:, 1:3, :], in1=MA[:, :, :], op=sub)
    V.tensor_tensor(out=OUT[:, 3:7:3, :], in0=OUT[:, 3:7:3, :], in1=M2[:, :, :], op=sub)
    V.tensor_tensor(out=OUT[:, 5:8:2, :], in0=OUT[:, 5:8:2, :], in1=M3[:, :, :], op=sub)

    nc.sync.dma_start(out=Op[t, :, :, :], in_=OUT[:, :, :])


# ---------------------------------------------------------------- runner

_NC = None


def _get_nc():
    global _NC
    if _NC is None:
        _NC = _build_nc()
    return _NC


def run_shards(shards, trace=False):
    """shards: list of 8 plane-major [9, S] fp32 arrays."""
    if trace:
        _install_ntff_hook()
    nc = _get_nc()
    in_maps = [{"F": sh} for sh in shards]
    res = run_bass_kernel_spmd(nc, in_maps, list(range(N_CORES)), trace=trace)
    outs = [res.results[k]["out"] for k in range(N_CORES)]
    return outs, res.exec_time_ns


# OUT tile slot order -> host plane (row-major); self-inverse (swaps 2<->3)
OUTPERM = [0, 1, 3, 2, 4, 5, 6, 7, 8]


def make_shards(F: np.ndarray):
    F = np.asarray(F)
    F2 = np.ascontiguousarray(F.reshape(N, 9).astype(np.float32, copy=False))
    total = N_CORES * S
    padded = np.empty((total, 9), dtype=np.float32)
    padded[:N] = F2
    padded[N:] = np.array([1, 0, 0, 0, 1, 0, 0, 0, 1], dtype=np.float32)
    base = padded.reshape(N_CORES, NT, P, L, 9).transpose(0, 1, 2, 4, 3)
    pm = np.ascontiguousarray(base[:, :, :, SIG, :])   # [8, NT, 128, 9, L]
    return [pm[k] for k in range(N_CORES)]


def unshard(outs):
    pm = np.stack(outs, axis=0)                        # [8, NT, 128, 9, L]
    sm = pm[:, :, :, OUTPERM, :].transpose(0, 1, 2, 4, 3).reshape(N_CORES * S, 9)
    return np.ascontiguousarray(sm[:N].reshape(N, 3, 3))


def kernel(F: np.ndarray) -> np.ndarray:
    shards = make_shards(F)
    outs, _ = run_shards(shards, trace=False)
    return unshard(outs)


# revision 20
# speedup vs baseline: 1.0064x; 1.0064x over previous
"""Trainium2 Bass kernel: batched Piola-Kirchhoff stress P = dW/dF.

Per-sample closed-form gradient of
  W = 8*I1 + 10*J^2 - 56*log(J) + 0.2*(I4^2 + I5^2) - 44
with C = F^T (elementwise*) F, G = diag(4, .5, .5):
  P = s*cof(F) + I1/I4 diag terms + I5 terms,  s = 20*J - 56/J.

Data-parallel over 8 NeuronCores. Host passes each core a PLANE-MAJOR
shard [9, S] so every on-chip access is unit-stride (strided DVE reads
are ~16x slower). Plane order sigma = [a,b,e,f,i,g,c,h,d] lets shifted
tensor_tensor ops compute 16 of the 18 minor products in 5 instructions.
"""
import sys, types

sys.path.insert(0, "/opt/trn_rl_repo")

import numpy as np
import concourse.bass as bass
import concourse.mybir as mybir
from concourse import tile
from concourse.vector_clock import ScopedClock
from concourse.bass_utils import run_bass_kernel_spmd

AF = mybir.ActivationFunctionType
OP = mybir.AluOpType
FP32 = mybir.dt.float32

N = 4_000_000
N_CORES = 8
P = 128
L = 489          # samples per partition per tile
NT = 8           # tiles per core
R = L * NT       # 3912 samples per partition
S = P * R        # 500,736 samples per core (padded total 4,005,888)
LN56 = float(np.log(56.0))

# slot k of the on-chip X tile holds input plane SIG[k] (row-major a..i)
#        a  b  e  f  i  g  c  h  d
SIG = [0, 1, 4, 5, 8, 6, 2, 7, 3]

# ---------------------------------------------------------------- patches


def _patch_tile():
    if getattr(tile.TileContext, "_pk_patched", False):
        return

    def patched(self, tick_clock, wait_clock):
        drain_inst = self.nc.sync.drain()
        wait_clock.add_sem_waits(
            drain_inst.ins, ScopedClock({None: tick_clock.global_clock})
        )
        si = drain_inst.ins.sync_info
        if si is not None and len(si.on_wait) > 1:
            waits = list(si.on_wait)
            drain_inst.ins.sync_info = mybir.SyncInfo(
                on_wait=[waits[0]], on_update=list(si.on_update)
            )
            for w in waits[1:]:
                extra = self.nc.sync.drain()
                extra.ins.sync_info = mybir.SyncInfo(on_wait=[w], on_update=[])
        self.nc.all_engine_barrier()
        popped = self.nc._tile_sem_poison_stack.pop()
        assert popped is self._sem_poison
        self.nc.clear_and_free_semaphores(list(self.sems.allocated().values()))
        self.nc.all_engine_barrier()

    tile.TileContext._drain_and_barrier = patched

    # walrus accepts at most ONE sem wait per instruction: hoist extras onto
    # standalone EventSemaphore ops just before, on the same engine.
    orig_lower = tile.TileContext._lower_ordered_insts

    def lower_split(self, ordered):
        nc = self.nc
        for bb_name, insts in list(ordered.items()):
            out = []
            for inst in insts:
                si = inst.sync_info
                if si is not None and len(si.on_wait) > 1:
                    waits = list(si.on_wait)
                    for w in waits[:-1]:
                        ev = mybir.InstEventSemaphore(
                            name=nc.get_next_instruction_name(), ins=[], outs=[]
                        )
                        ev.engine = inst.engine
                        ev.sync_info = mybir.SyncInfo(on_wait=[w], on_update=[])
                        out.append(ev)
                    inst.sync_info = mybir.SyncInfo(
                        on_wait=[waits[-1]], on_update=list(si.on_update)
                    )
                out.append(inst)
            ordered[bb_name] = out
        return orig_lower(self, ordered)

    tile.TileContext._lower_ordered_insts = lower_split
    tile.TileContext._pk_patched = True


def _install_ntff_hook():
    try:
        import antenv
        if "antenv.axon_hooks" not in sys.modules:
            mod = types.ModuleType("antenv.axon_hooks")
            mod._hook = None
            mod.set_axon_ntff_profile_hook = lambda h: setattr(mod, "_hook", h)
            mod.get_axon_ntff_profile_hook = lambda: mod._hook
            sys.modules["antenv.axon_hooks"] = mod
            antenv.axon_hooks = mod
        from trn_agent_boot.trn_boot import _ntff_profile_via_ctypes
        sys.modules["antenv.axon_hooks"].set_axon_ntff_profile_hook(
            _ntff_profile_via_ctypes("/opt/axon/libaxon_pjrt.so")
        )
        import concourse.bass_utils as bu
        bu.upload_artifacts = lambda tmpdir: tmpdir
    except Exception as e:
        print(f"ntff hook install failed: {e}", file=sys.stderr)


# ---------------------------------------------------------------- program




def _b3(ap2d, nplanes):
    """[P, L] view of a [P,1,L] tile -> broadcast AP [P, nplanes, L]."""
    t = ap2d.tensor
    return t[:, :, :].to_broadcast((P, nplanes, L))

def _build_nc():
    _patch_tile()
    nc = bass.Bass()
    _c = nc.alloc_sbuf_tensor("const-ln56", [128, 1], FP32)
    nc.gpsimd.memset(_c.ap(), LN56)
    nc.const_aps.aps[(FP32, LN56)] = _c.ap()
    nc.all_engine_barrier()

    # pre-tiled host layout: [NT, 128, 9, L] (X-slot plane order baked on host)
    Fd = nc.dram_tensor("F", [NT, P, 9, L], FP32, kind="ExternalInput")
    Od = nc.dram_tensor("out", [NT, P, 9, L], FP32, kind="ExternalOutput")
    Fp, Op = Fd, Od

    with tile.TileContext(nc) as tc:
        with (
            tc.tile_pool(name="io", bufs=4) as iop,
            tc.tile_pool(name="mid", bufs=1) as midp,
            tc.tile_pool(name="mid2", bufs=1) as midp2,
            tc.tile_pool(name="sm", bufs=1) as smp,
        ):
            for t in range(NT):
                _emit_tile(nc, iop, midp, midp2, smp, Fp, Op, t)
    return nc


def _emit_tile(nc, iop, midp, midp2, smp, Fp, Op, t):
    mul, add, sub = OP.mult, OP.add, OP.subtract
    sl = slice(t * L, (t + 1) * L)

    X = iop.tile([P, 9, L], FP32, name="x", tag="x")
    nc.sync.dma_start(out=X[:, :, :], in_=Fp[t, :, :, :])

    def mtile(name, planes):
        return midp.tile([P, planes, L], FP32, name=name, tag=name)

    def stile(name):
        t = smp.tile([P, 1, L], FP32, name=name, tag=name)
        return t[:, 0, :]

    V = nc.vector

    # --- 18 minor products (shift-batched), PROD slots:
    # 0:cg 1:ch 2:dh | 3:ae 4:bf 5:ei 6:fg | 7:cd | 8:af 9:bi 10:eg |
    # 11:ai 12:bg 13:ce 14:fh 15:di | 16:ah 17:bd
    PR = midp2.tile([P, 18, L], FP32, name="prod", tag="prod")
    V.tensor_tensor(out=PR[:, 0:3, :], in0=X[:, 5:8, :], in1=X[:, 6:9, :], op=mul)
    V.tensor_tensor(out=PR[:, 3:7, :], in0=X[:, 0:4, :], in1=X[:, 2:6, :], op=mul)
    V.tensor_tensor(out=PR[:, 7, :], in0=X[:, 6, :], in1=X[:, 8, :], op=mul)
    V.tensor_tensor(out=PR[:, 8:11, :], in0=X[:, 0:3, :], in1=X[:, 3:6, :], op=mul)
    V.tensor_tensor(out=PR[:, 11:16, :], in0=X[:, 0:5, :], in1=X[:, 4:9, :], op=mul)
    V.tensor_tensor(out=PR[:, 16:18, :], in0=X[:, 0:2, :], in1=X[:, 7:9, :], op=mul)

    # early copies of late-needed products so PROD can retire after cofactors
    BDCF = mtile("bdcf", 3)          # (bd, cg, 8*fh)
    nc.scalar.activation(out=BDCF[:, 0, :], in_=PR[:, 17, :], func=AF.Copy)
    nc.scalar.activation(out=BDCF[:, 1, :], in_=PR[:, 0, :], func=AF.Copy)
    nc.scalar.activation(out=BDCF[:, 2, :], in_=PR[:, 14, :], func=AF.Copy, scale=8.0)

    # --- cofactors, row-major: 0:c00 1:c01 2:c02 3:c10 4:c11 5:c12 6:c20 7:c21 8:c22
    C = mtile("cof", 9)
    # C order: 0:c00 1:c01 2:c10 3:c02 4:c11 5:c12 6:c20 7:c21 8:c22
    V.tensor_tensor(out=C[:, 0:2, :], in0=PR[:, 5:7, :], in1=PR[:, 14:16, :], op=sub)
    V.tensor_tensor(out=C[:, 2:4, :], in0=PR[:, 1:3, :], in1=PR[:, 9:11, :], op=sub)
    V.tensor_tensor(out=C[:, 4, :], in0=PR[:, 11, :], in1=PR[:, 0, :], op=sub)
    V.tensor_tensor(out=C[:, 5, :], in0=PR[:, 12, :], in1=PR[:, 16, :], op=sub)
    V.tensor_tensor(out=C[:, 6, :], in0=PR[:, 4, :], in1=PR[:, 13, :], op=sub)
    V.tensor_tensor(out=C[:, 7, :], in0=PR[:, 7, :], in1=PR[:, 8, :], op=sub)
    V.tensor_tensor(out=C[:, 8, :], in0=PR[:, 3, :], in1=PR[:, 17, :], op=sub)

    # --- J = a*c00 + b*c01 + c*c02
    TAB = mtile("tab", 2)
    V.tensor_tensor(out=TAB[:, :, :], in0=X[:, 0:2, :], in1=C[:, 0:2, :], op=mul)
    tcp = stile("tcp")
    V.tensor_tensor(out=tcp, in0=X[:, 6, :], in1=C[:, 3, :], op=mul)
    j1 = stile("j1")
    V.tensor_tensor(out=j1, in0=TAB[:, 0, :], in1=TAB[:, 1, :], op=add)
    J = stile("J")
    V.tensor_tensor(out=J, in0=j1, in1=tcp, op=add)

    # --- s = 20*J - 56/J  (56/J = exp(-lnJ + ln56) on ACT)
    lnJ = stile("lnJ")
    nc.scalar.activation(out=lnJ, in_=J, func=AF.Ln, scale=1.0, bias=0.0)
    rec56 = stile("rec56")
    nc.scalar.activation(out=rec56, in_=lnJ, func=AF.Exp, scale=-1.0, bias=LN56)
    s = stile("s")
    V.scalar_tensor_tensor(out=s, in0=J, scalar=20.0, in1=rec56, op0=mul, op1=sub)

    # --- I5: q5x = 8*(ei^2-fh^2) + (ai^2-cg^2) + (ae^2-bd^2);  0.2*q5x = 0.4*I5
    SQ1 = mtile("sq1", 4)   # (ae2, bf2, ei2, fg2)
    nc.scalar.activation(out=SQ1[:, :, :], in_=PR[:, 3:7, :], func=AF.Square)
    SQ2 = mtile("sq2", 4)   # (ai2, bg2, ce2, fh2)
    nc.scalar.activation(out=SQ2[:, :, :], in_=PR[:, 11:15, :], func=AF.Square)
    SQBC = mtile("sqbc", 2)          # (sq_bd, sq_cg)
    nc.scalar.activation(out=SQBC[:, :, :], in_=BDCF[:, 0:2, :], func=AF.Square)
    sq_bd = SQBC[:, 0, :]
    sq_cg = SQBC[:, 1, :]
    t0 = stile("t0")
    V.tensor_tensor(out=t0, in0=SQ1[:, 2, :], in1=SQ2[:, 3, :], op=sub)
    t1 = stile("t1")
    V.tensor_tensor(out=t1, in0=SQ2[:, 0, :], in1=sq_cg, op=sub)
    t2 = stile("t2")
    V.tensor_tensor(out=t2, in0=SQ1[:, 0, :], in1=sq_bd, op=sub)
    u = stile("u")
    V.tensor_tensor(out=u, in0=t1, in1=t2, op=add)
    q5x = stile("q5x")
    V.scalar_tensor_tensor(out=q5x, in0=t0, scalar=8.0, in1=u, op0=mul, op1=add)

    # --- diag helpers: squares of a,e,i  (X slots 0,2,4)
    SQX = mtile("sqx", 5)   # (a2, b2, e2, f2, i2)
    nc.scalar.activation(out=SQX[:, :, :], in_=X[:, 0:5, :], func=AF.Square)
    YS = mtile("ys", 3)     # (y, y11, y22)
    V.tensor_tensor(out=YS[:, 0, :], in0=SQX[:, 2, :], in1=SQX[:, 4, :], op=add)
    V.scalar_tensor_tensor(out=YS[:, 1, :], in0=SQX[:, 4, :], scalar=8.0,
                           in1=SQX[:, 0, :], op0=mul, op1=add)
    V.scalar_tensor_tensor(out=YS[:, 2, :], in0=SQX[:, 2, :], scalar=8.0,
                           in1=SQX[:, 0, :], op0=mul, op1=add)
    I4x = stile("I4x")
    V.scalar_tensor_tensor(out=I4x, in0=SQX[:, 0, :], scalar=8.0,
                           in1=YS[:, 0, :], op0=mul, op1=add)

    # --- v = (0.2*ys)*q5x ; w = wI4*I4x + v ; za = (w+16)*(a,e,i)
    q5x3 = _b3(q5x, 3)
    V.scalar_tensor_tensor(out=YS[:, :, :], in0=YS[:, :, :], scalar=0.2,
                           in1=q5x3, op0=mul, op1=mul)
    V.scalar_tensor_tensor(out=YS[:, 0, :], in0=I4x, scalar=1.6,
                           in1=YS[:, 0, :], op0=mul, op1=add)
    I4x2 = _b3(I4x, 2)
    V.scalar_tensor_tensor(out=YS[:, 1:3, :], in0=I4x2, scalar=0.2,
                           in1=YS[:, 1:3, :], op0=mul, op1=add)
    ZA = YS
    V.scalar_tensor_tensor(out=ZA[:, :, :], in0=ZA[:, :, :], scalar=16.0,
                           in1=X[:, 0:5:2, :], op0=add, op1=mul)

    # --- OUT = s * cof (all 9), finals applied in place
    OUT = iop.tile([P, 9, L], FP32, name="outt", tag="x")
    s9 = _b3(s, 9)
    V.tensor_tensor(out=OUT[:, :, :], in0=s9, in1=C[:, :, :], op=mul)
    # diag: OUT[0,4,8] += ZA
    V.tensor_tensor(out=OUT[:, 0:9:4, :], in0=OUT[:, 0:9:4, :], in1=ZA[:, :, :], op=add)

    # --- off-diagonals
    Q3 = mtile("q3", 3)              # (q5bd, q5cg, q5fh8) = 0.2*q5x*(bd, cg, 8fh)
    q5x3b = _b3(q5x, 3)
    V.scalar_tensor_tensor(out=Q3[:, :, :], in0=q5x3b, scalar=0.2,
                           in1=BDCF[:, :, :], op0=mul, op1=mul)
    q5bd = Q3[:, 0, :]
    q5cg = Q3[:, 1, :]
    q5fh8 = Q3[:, 2, :]

    MA = mtile("ma", 2)     # (m01, m10) = q5bd * (d, b)
    V.tensor_tensor(out=MA[:, 0, :], in0=q5bd, in1=X[:, 8, :], op=mul)   # *d
    V.tensor_tensor(out=MA[:, 1, :], in0=q5bd, in1=X[:, 1, :], op=mul)   # *b
    M2 = mtile("m2", 2)     # (m02, m20) = q5cg * (g, c) ; g=X5 c=X6
    q5cg2 = Q3[:, 1:2, :].to_broadcast((P, 2, L))
    V.tensor_tensor(out=M2[:, :, :], in0=q5cg2, in1=X[:, 5:7, :], op=mul)
    M3 = mtile("m3", 2)     # (m12, m21) = q5fh8 * (h, f)
    V.tensor_tensor(out=M3[:, 0, :], in0=q5fh8, in1=X[:, 7, :], op=mul)  # *h
    V.tensor_tensor(out=M3[:, 1, :], in0=q5fh8, in1=X[:, 3, :], op=mul)  # *f

    # P01,P10 -> OUT[1:3]; P02@3, P20@6 stride 3; P12@5, P21@7 stride 2
    V.tensor_tensor(out=OUT[:, 1:3, :], in0=OUT[:, 1:3, :], in1=MA[:, :, :], op=sub)
    V.tensor_tensor(out=OUT[:, 3:7:3, :], in0=OUT[:, 3:7:3, :], in1=M2[:, :, :], op=sub)
    V.tensor_tensor(out=OUT[:, 5:8:2, :], in0=OUT[:, 5:8:2, :], in1=M3[:, :, :], op=sub)

    nc.sync.dma_start(out=Op[t, :, :, :], in_=OUT[:, :, :])


# ---------------------------------------------------------------- runner

_NC = None


def _get_nc():
    global _NC
    if _NC is None:
        _NC = _build_nc()
    return _NC


def run_shards(shards, trace=False):
    """shards: list of 8 plane-major [9, S] fp32 arrays."""
    if trace:
        _install_ntff_hook()
    nc = _get_nc()
    in_maps = [{"F": sh} for sh in shards]
    res = run_bass_kernel_spmd(nc, in_maps, list(range(N_CORES)), trace=trace)
    outs = [res.results[k]["out"] for k in range(N_CORES)]
    return outs, res.exec_time_ns


# OUT tile slot order -> host plane (row-major); self-inverse (swaps 2<->3)
OUTPERM = [0, 1, 3, 2, 4, 5, 6, 7, 8]


def make_shards(F: np.ndarray):
    F = np.asarray(F)
    F2 = np.ascontiguousarray(F.reshape(N, 9).astype(np.float32, copy=False))
    total = N_CORES * S
    padded = np.empty((total, 9), dtype=np.float32)
    padded[:N] = F2
    padded[N:] = np.array([1, 0, 0, 0, 1, 0, 0, 0, 1], dtype=np.float32)
    base = padded.reshape(N_CORES, NT, P, L, 9).transpose(0, 1, 2, 4, 3)
    pm = np.ascontiguousarray(base[:, :, :, SIG, :])   # [8, NT, 128, 9, L]
    return [pm[k] for k in range(N_CORES)]


def unshard(outs):
    pm = np.stack(outs, axis=0)                        # [8, NT, 128, 9, L]
    sm = pm[:, :, :, OUTPERM, :].transpose(0, 1, 2, 4, 3).reshape(N_CORES * S, 9)
    return np.ascontiguousarray(sm[:N].reshape(N, 3, 3))


def kernel(F: np.ndarray) -> np.ndarray:
    shards = make_shards(F)
    outs, _ = run_shards(shards, trace=False)
    return unshard(outs)


# revision 21
# speedup vs baseline: 1.0092x; 1.0028x over previous
"""Trainium2 Bass kernel: batched Piola-Kirchhoff stress P = dW/dF.

Per-sample closed-form gradient of
  W = 8*I1 + 10*J^2 - 56*log(J) + 0.2*(I4^2 + I5^2) - 44
with C = F^T (elementwise*) F, G = diag(4, .5, .5):
  P = s*cof(F) + I1/I4 diag terms + I5 terms,  s = 20*J - 56/J.

Data-parallel over 8 NeuronCores. Host passes each core a PLANE-MAJOR
shard [9, S] so every on-chip access is unit-stride (strided DVE reads
are ~16x slower). Plane order sigma = [a,b,e,f,i,g,c,h,d] lets shifted
tensor_tensor ops compute 16 of the 18 minor products in 5 instructions.
"""
import sys, types

sys.path.insert(0, "/opt/trn_rl_repo")

import numpy as np
import concourse.bass as bass
import concourse.mybir as mybir
from concourse import tile
from concourse.vector_clock import ScopedClock
from concourse.bass_utils import run_bass_kernel_spmd

AF = mybir.ActivationFunctionType
OP = mybir.AluOpType
FP32 = mybir.dt.float32

N = 4_000_000
N_CORES = 8
P = 128
L = 489          # samples per partition per tile
NT = 8           # tiles per core
R = L * NT       # 3912 samples per partition
S = P * R        # 500,736 samples per core (padded total 4,005,888)
LN56 = float(np.log(56.0))

# slot k of the on-chip X tile holds input plane SIG[k] (row-major a..i)
#        a  b  e  f  i  g  c  h  d
SIG = [0, 1, 4, 5, 8, 6, 2, 7, 3]

# ---------------------------------------------------------------- patches


def _patch_tile():
    if getattr(tile.TileContext, "_pk_patched", False):
        return

    def patched(self, tick_clock, wait_clock):
        drain_inst = self.nc.sync.drain()
        wait_clock.add_sem_waits(
            drain_inst.ins, ScopedClock({None: tick_clock.global_clock})
        )
        si = drain_inst.ins.sync_info
        if si is not None and len(si.on_wait) > 1:
            waits = list(si.on_wait)
            drain_inst.ins.sync_info = mybir.SyncInfo(
                on_wait=[waits[0]], on_update=list(si.on_update)
            )
            for w in waits[1:]:
                extra = self.nc.sync.drain()
                extra.ins.sync_info = mybir.SyncInfo(on_wait=[w], on_update=[])
        self.nc.all_engine_barrier()
        popped = self.nc._tile_sem_poison_stack.pop()
        assert popped is self._sem_poison
        self.nc.clear_and_free_semaphores(list(self.sems.allocated().values()))
        self.nc.all_engine_barrier()

    tile.TileContext._drain_and_barrier = patched

    # walrus accepts at most ONE sem wait per instruction: hoist extras onto
    # standalone EventSemaphore ops just before, on the same engine.
    orig_lower = tile.TileContext._lower_ordered_insts

    def lower_split(self, ordered):
        nc = self.nc
        for bb_name, insts in list(ordered.items()):
            out = []
            for inst in insts:
                si = inst.sync_info
                if si is not None and len(si.on_wait) > 1:
                    waits = list(si.on_wait)
                    for w in waits[:-1]:
                        ev = mybir.InstEventSemaphore(
                            name=nc.get_next_instruction_name(), ins=[], outs=[]
                        )
                        ev.engine = inst.engine
                        ev.sync_info = mybir.SyncInfo(on_wait=[w], on_update=[])
                        out.append(ev)
                    inst.sync_info = mybir.SyncInfo(
                        on_wait=[waits[-1]], on_update=list(si.on_update)
                    )
                out.append(inst)
            ordered[bb_name] = out
        return orig_lower(self, ordered)

    tile.TileContext._lower_ordered_insts = lower_split
    tile.TileContext._pk_patched = True


def _install_ntff_hook():
    try:
        import antenv
        if "antenv.axon_hooks" not in sys.modules:
            mod = types.ModuleType("antenv.axon_hooks")
            mod._hook = None
            mod.set_axon_ntff_profile_hook = lambda h: setattr(mod, "_hook", h)
            mod.get_axon_ntff_profile_hook = lambda: mod._hook
            sys.modules["antenv.axon_hooks"] = mod
            antenv.axon_hooks = mod
        from trn_agent_boot.trn_boot import _ntff_profile_via_ctypes
        sys.modules["antenv.axon_hooks"].set_axon_ntff_profile_hook(
            _ntff_profile_via_ctypes("/opt/axon/libaxon_pjrt.so")
        )
        import concourse.bass_utils as bu
        bu.upload_artifacts = lambda tmpdir: tmpdir
    except Exception as e:
        print(f"ntff hook install failed: {e}", file=sys.stderr)


# ---------------------------------------------------------------- program




def _b3(ap2d, nplanes):
    """[P, L] view of a [P,1,L] tile -> broadcast AP [P, nplanes, L]."""
    t = ap2d.tensor
    return t[:, :, :].to_broadcast((P, nplanes, L))

def _build_nc():
    _patch_tile()
    nc = bass.Bass()
    _c = nc.alloc_sbuf_tensor("const-ln56", [128, 1], FP32)
    nc.gpsimd.memset(_c.ap(), LN56)
    nc.const_aps.aps[(FP32, LN56)] = _c.ap()
    nc.all_engine_barrier()

    # pre-tiled host layout: [NT, 128, 9, L] (X-slot plane order baked on host)
    Fd = nc.dram_tensor("F", [NT, P, 9, L], FP32, kind="ExternalInput")
    Od = nc.dram_tensor("out", [NT, P, 9, L], FP32, kind="ExternalOutput")
    Fp, Op = Fd, Od

    with tile.TileContext(nc) as tc:
        with (
            tc.tile_pool(name="io", bufs=3) as iop,
            tc.tile_pool(name="mid", bufs=1) as midp,
            tc.tile_pool(name="mid2", bufs=1) as midp2,
            tc.tile_pool(name="sm", bufs=1) as smp,
        ):
            for t in range(NT):
                _emit_tile(nc, iop, midp, midp2, smp, Fp, Op, t)
    return nc


def _emit_tile(nc, iop, midp, midp2, smp, Fp, Op, t):
    mul, add, sub = OP.mult, OP.add, OP.subtract
    sl = slice(t * L, (t + 1) * L)

    X = iop.tile([P, 9, L], FP32, name="x", tag="x")
    nc.sync.dma_start(out=X[:, :, :], in_=Fp[t, :, :, :])

    def mtile(name, planes):
        return midp.tile([P, planes, L], FP32, name=name, tag=name)

    def stile(name):
        t = smp.tile([P, 1, L], FP32, name=name, tag=name)
        return t[:, 0, :]

    V = nc.vector

    # --- 18 minor products (shift-batched), PROD slots:
    # 0:cg 1:ch 2:dh | 3:ae 4:bf 5:ei 6:fg | 7:cd | 8:af 9:bi 10:eg |
    # 11:ai 12:bg 13:ce 14:fh 15:di | 16:ah 17:bd
    PR = midp2.tile([P, 18, L], FP32, name="prod", tag="prod")
    V.tensor_tensor(out=PR[:, 0:3, :], in0=X[:, 5:8, :], in1=X[:, 6:9, :], op=mul)
    V.tensor_tensor(out=PR[:, 3:7, :], in0=X[:, 0:4, :], in1=X[:, 2:6, :], op=mul)
    V.tensor_tensor(out=PR[:, 7, :], in0=X[:, 6, :], in1=X[:, 8, :], op=mul)
    V.tensor_tensor(out=PR[:, 8:11, :], in0=X[:, 0:3, :], in1=X[:, 3:6, :], op=mul)
    V.tensor_tensor(out=PR[:, 11:16, :], in0=X[:, 0:5, :], in1=X[:, 4:9, :], op=mul)
    V.tensor_tensor(out=PR[:, 16:18, :], in0=X[:, 0:2, :], in1=X[:, 7:9, :], op=mul)

    # early copies of late-needed products so PROD can retire after cofactors
    BDCF = mtile("bdcf", 3)          # (bd, cg, 8*fh)
    nc.scalar.activation(out=BDCF[:, 0, :], in_=PR[:, 17, :], func=AF.Copy)
    nc.scalar.activation(out=BDCF[:, 1, :], in_=PR[:, 0, :], func=AF.Copy)
    nc.scalar.activation(out=BDCF[:, 2, :], in_=PR[:, 14, :], func=AF.Copy, scale=8.0)

    # --- cofactors, row-major: 0:c00 1:c01 2:c02 3:c10 4:c11 5:c12 6:c20 7:c21 8:c22
    C = mtile("cof", 9)
    # C order: 0:c00 1:c01 2:c10 3:c02 4:c11 5:c12 6:c20 7:c21 8:c22
    V.tensor_tensor(out=C[:, 0:2, :], in0=PR[:, 5:7, :], in1=PR[:, 14:16, :], op=sub)
    V.tensor_tensor(out=C[:, 2:4, :], in0=PR[:, 1:3, :], in1=PR[:, 9:11, :], op=sub)
    V.tensor_tensor(out=C[:, 4, :], in0=PR[:, 11, :], in1=PR[:, 0, :], op=sub)
    V.tensor_tensor(out=C[:, 5, :], in0=PR[:, 12, :], in1=PR[:, 16, :], op=sub)
    V.tensor_tensor(out=C[:, 6, :], in0=PR[:, 4, :], in1=PR[:, 13, :], op=sub)
    V.tensor_tensor(out=C[:, 7, :], in0=PR[:, 7, :], in1=PR[:, 8, :], op=sub)
    V.tensor_tensor(out=C[:, 8, :], in0=PR[:, 3, :], in1=PR[:, 17, :], op=sub)

    # --- J = a*c00 + b*c01 + c*c02
    TAB = mtile("tab", 2)
    V.tensor_tensor(out=TAB[:, :, :], in0=X[:, 0:2, :], in1=C[:, 0:2, :], op=mul)
    tcp = stile("tcp")
    V.tensor_tensor(out=tcp, in0=X[:, 6, :], in1=C[:, 3, :], op=mul)
    j1 = stile("j1")
    V.tensor_tensor(out=j1, in0=TAB[:, 0, :], in1=TAB[:, 1, :], op=add)
    J = stile("J")
    V.tensor_tensor(out=J, in0=j1, in1=tcp, op=add)

    # --- s = 20*J - 56/J  (56/J = exp(-lnJ + ln56) on ACT)
    lnJ = stile("lnJ")
    nc.scalar.activation(out=lnJ, in_=J, func=AF.Ln, scale=1.0, bias=0.0)
    rec56 = stile("rec56")
    nc.scalar.activation(out=rec56, in_=lnJ, func=AF.Exp, scale=-1.0, bias=LN56)
    s = stile("s")
    V.scalar_tensor_tensor(out=s, in0=J, scalar=20.0, in1=rec56, op0=mul, op1=sub)

    # --- I5: q5x = 8*(ei^2-fh^2) + (ai^2-cg^2) + (ae^2-bd^2);  0.2*q5x = 0.4*I5
    SQ1 = mtile("sq1", 4)   # (ae2, bf2, ei2, fg2)
    nc.scalar.activation(out=SQ1[:, :, :], in_=PR[:, 3:7, :], func=AF.Square)
    SQ2 = mtile("sq2", 4)   # (ai2, bg2, ce2, fh2)
    nc.scalar.activation(out=SQ2[:, :, :], in_=PR[:, 11:15, :], func=AF.Square)
    SQBC = mtile("sqbc", 2)          # (sq_bd, sq_cg)
    nc.scalar.activation(out=SQBC[:, :, :], in_=BDCF[:, 0:2, :], func=AF.Square)
    sq_bd = SQBC[:, 0, :]
    sq_cg = SQBC[:, 1, :]
    t0 = stile("t0")
    V.tensor_tensor(out=t0, in0=SQ1[:, 2, :], in1=SQ2[:, 3, :], op=sub)
    t1 = stile("t1")
    V.tensor_tensor(out=t1, in0=SQ2[:, 0, :], in1=sq_cg, op=sub)
    t2 = stile("t2")
    V.tensor_tensor(out=t2, in0=SQ1[:, 0, :], in1=sq_bd, op=sub)
    u = stile("u")
    V.tensor_tensor(out=u, in0=t1, in1=t2, op=add)
    q5x = stile("q5x")
    V.scalar_tensor_tensor(out=q5x, in0=t0, scalar=8.0, in1=u, op0=mul, op1=add)

    # --- diag helpers: squares of a,e,i  (X slots 0,2,4)
    SQX = mtile("sqx", 5)   # (a2, b2, e2, f2, i2)
    nc.scalar.activation(out=SQX[:, :, :], in_=X[:, 0:5, :], func=AF.Square)
    YS = mtile("ys", 3)     # (y, y11, y22)
    V.tensor_tensor(out=YS[:, 0, :], in0=SQX[:, 2, :], in1=SQX[:, 4, :], op=add)
    V.scalar_tensor_tensor(out=YS[:, 1, :], in0=SQX[:, 4, :], scalar=8.0,
                           in1=SQX[:, 0, :], op0=mul, op1=add)
    V.scalar_tensor_tensor(out=YS[:, 2, :], in0=SQX[:, 2, :], scalar=8.0,
                           in1=SQX[:, 0, :], op0=mul, op1=add)
    I4x = stile("I4x")
    V.scalar_tensor_tensor(out=I4x, in0=SQX[:, 0, :], scalar=8.0,
                           in1=YS[:, 0, :], op0=mul, op1=add)

    # --- v = (0.2*ys)*q5x ; w = wI4*I4x + v ; za = (w+16)*(a,e,i)
    q5x3 = _b3(q5x, 3)
    V.scalar_tensor_tensor(out=YS[:, :, :], in0=YS[:, :, :], scalar=0.2,
                           in1=q5x3, op0=mul, op1=mul)
    V.scalar_tensor_tensor(out=YS[:, 0, :], in0=I4x, scalar=1.6,
                           in1=YS[:, 0, :], op0=mul, op1=add)
    I4x2 = _b3(I4x, 2)
    V.scalar_tensor_tensor(out=YS[:, 1:3, :], in0=I4x2, scalar=0.2,
                           in1=YS[:, 1:3, :], op0=mul, op1=add)
    ZA = YS
    V.scalar_tensor_tensor(out=ZA[:, :, :], in0=ZA[:, :, :], scalar=16.0,
                           in1=X[:, 0:5:2, :], op0=add, op1=mul)

    # --- OUT = s * cof (all 9), finals applied in place
    OUT = iop.tile([P, 9, L], FP32, name="outt", tag="x")
    s9 = _b3(s, 9)
    V.tensor_tensor(out=OUT[:, :, :], in0=s9, in1=C[:, :, :], op=mul)
    # diag: OUT[0,4,8] += ZA
    V.tensor_tensor(out=OUT[:, 0:9:4, :], in0=OUT[:, 0:9:4, :], in1=ZA[:, :, :], op=add)

    # --- off-diagonals
    Q3 = mtile("q3", 3)              # (q5bd, q5cg, q5fh8) = 0.2*q5x*(bd, cg, 8fh)
    q5x3b = _b3(q5x, 3)
    V.scalar_tensor_tensor(out=Q3[:, :, :], in0=q5x3b, scalar=0.2,
                           in1=BDCF[:, :, :], op0=mul, op1=mul)
    q5bd = Q3[:, 0, :]
    q5cg = Q3[:, 1, :]
    q5fh8 = Q3[:, 2, :]

    MA = mtile("ma", 2)     # (m01, m10) = q5bd * (d, b)
    V.tensor_tensor(out=MA[:, 0, :], in0=q5bd, in1=X[:, 8, :], op=mul)   # *d
    V.tensor_tensor(out=MA[:, 1, :], in0=q5bd, in1=X[:, 1, :], op=mul)   # *b
    M2 = mtile("m2", 2)     # (m02, m20) = q5cg * (g, c) ; g=X5 c=X6
    q5cg2 = Q3[:, 1:2, :].to_broadcast((P, 2, L))
    V.tensor_tensor(out=M2[:, :, :], in0=q5cg2, in1=X[:, 5:7, :], op=mul)
    M3 = mtile("m3", 2)     # (m12, m21) = q5fh8 * (h, f)
    V.tensor_tensor(out=M3[:, 0, :], in0=q5fh8, in1=X[:, 7, :], op=mul)  # *h
    V.tensor_tensor(out=M3[:, 1, :], in0=q5fh8, in1=X[:, 3, :], op=mul)  # *f

    # P01,P10 -> OUT[1:3]; P02@3, P20@6 stride 3; P12@5, P21@7 stride 2
    V.tensor_tensor(out=OUT[:, 1:3, :], in0=OUT[:, 1:3, :], in1=MA[:, :, :], op=sub)
    V.tensor_tensor(out=OUT[:, 3:7:3, :], in0=OUT[:, 3:7:3, :], in1=M2[:, :, :], op=sub)
    V.tensor_tensor(out=OUT[:, 5:8:2, :], in0=OUT[:, 5:8:2, :], in1=M3[:, :, :], op=sub)

    nc.sync.dma_start(out=Op[t, :, :, :], in_=OUT[:, :, :])


# ---------------------------------------------------------------- runner

_NC = None


def _get_nc():
    global _NC
    if _NC is None:
        _NC = _build_nc()
    return _NC


def run_shards(shards, trace=False):
    """shards: list of 8 plane-major [9, S] fp32 arrays."""
    if trace:
        _install_ntff_hook()
    nc = _get_nc()
    in_maps = [{"F": sh} for sh in shards]
    res = run_bass_kernel_spmd(nc, in_maps, list(range(N_CORES)), trace=trace)
    outs = [res.results[k]["out"] for k in range(N_CORES)]
    return outs, res.exec_time_ns


# OUT tile slot order -> host plane (row-major); self-inverse (swaps 2<->3)
OUTPERM = [0, 1, 3, 2, 4, 5, 6, 7, 8]


def make_shards(F: np.ndarray):
    F = np.asarray(F)
    F2 = np.ascontiguousarray(F.reshape(N, 9).astype(np.float32, copy=False))
    total = N_CORES * S
    padded = np.empty((total, 9), dtype=np.float32)
    padded[:N] = F2
    padded[N:] = np.array([1, 0, 0, 0, 1, 0, 0, 0, 1], dtype=np.float32)
    base = padded.reshape(N_CORES, NT, P, L, 9).transpose(0, 1, 2, 4, 3)
    pm = np.ascontiguousarray(base[:, :, :, SIG, :])   # [8, NT, 128, 9, L]
    return [pm[k] for k in range(N_CORES)]


def unshard(outs):
    pm = np.stack(outs, axis=0)                        # [8, NT, 128, 9, L]
    sm = pm[:, :, :, OUTPERM, :].transpose(0, 1, 2, 4, 3).reshape(N_CORES * S, 9)
    return np.ascontiguousarray(sm[:N].reshape(N, 3, 3))


def kernel(F: np.ndarray) -> np.ndarray:
    shards = make_shards(F)
    outs, _ = run_shards(shards, trace=False)
    return unshard(outs)
